# revision 61
# baseline (speedup 1.0000x reference)
"""Multi-head causal self-attention on 8 TRN2 NeuronCores.

Problem: B=2, T=4096, D=512, H=8 heads (hd=64), fp32 in/out.

Sharding: core c in 0..7 handles batch b = c//4 and head pair g = c%4
(heads 2g, 2g+1 -> D-slice [128g, 128g+128)). Each core computes
    partial_out = concat_h( softmax(causal(Q_h K_h^T / 8)) V_h ) @ W_O[slice]
for its two heads; the host sums the 4 partials per batch and adds b_O.

On-core dataflow (all matmul operands bf16, f32 PSUM accumulation):
  - Slices are processed in DESCENDING q order (7,6,...,0): slice 7 needs
    every key block, so all QKV projection emissions interleave between its
    32 score blocks, exp work is front-loaded for ScalarE, and the kernel
    ends on the smallest slice where the output tail is cheapest.
  - X^T streams in as 32 [128,512] tiles over three DMA queues in
    consumption order (a single DMA ~50GB/s on one engine - parallelism
    needs many in-flight transfers).
  - Scores are computed transposed, S^T[k-block, q], contraction over the
    64-dim head axis. One PSUM tile per key block holds BOTH heads
    (h0 at cols 0:512, h1 at 512:1024): the pair are same-engine writes
    with no semaphore between them, so the PE's 64-deep reorder window
    runs the two row-disjoint (tile_position 0/64) matmuls CONCURRENTLY
    (~4ns apart, measured) - the K=64 half-array waste cancels out.
  - exp() runs on ScalarE straight out of PSUM with the 1/8 scale folded
    in, one call per block covering both heads; the diagonal 128x128
    subtile is masked by accumulating -1e9 upper-triangle via an identity
    matmul before the exp. The exp ACT table is preloaded at t=0.
  - Z^T_aug[65, q] accumulates P^T-block x V_aug over key blocks in PSUM;
    the V ones-columns make the row-sum L fall out of the PV matmul for
    free; b_V is added via a pre-replicated SBUF tile fused into the DVE
    evacuation. The scores->exp->PV chain is pipelined one block deep and
    ACROSS slice boundaries: the previous slice's last PV, normalisation
    and an older O-projection are emitted under the next slice's first
    blocks.
  - Normalisation: evacuate Z_aug, 1/L on a [128,8] partition-spread via
    one DRAM round-trip shared by both heads, step-0 partition-broadcast
    DMA read back, one DVE multiply to bf16; head B shifted to partitions
    64..127 by an SBUF->SBUF DMA; O-projection (K=128 head-pair stacked)
    emitted a slice later so the chain never stalls the PE.
  - Tail (last-processed slice 0): per-head O-projection (K=64, head-B
    weights re-homed at partitions 0..63) scaled at PSUM eviction by the
    column-major partition-spread reciprocal directly - no broadcast
    bounce, no shift, ~halving the end-of-kernel serial chain.
"""

import numpy as np

import concourse.bass as bass
import concourse.mybir as mybir
from concourse.tile import TileContext
from concourse.bass_utils import run_bass_kernel_spmd

try:
    import ml_dtypes

    _BF16 = ml_dtypes.bfloat16
except ImportError:  # pragma: no cover
    _BF16 = None

F32 = mybir.dt.float32
BF16 = mybir.dt.bfloat16
I16 = mybir.dt.int16

# Schraudolph exp for bf16 via int16 bit-trick (DVE round-to-nearest
# convert): bf16_bits(exp(s/8)) ~= rint(s * (2^7/ln2)*0.125 + (127*2^7 - 7))
# (~1.8% RMS rel err; the -7 centers the mantissa-interpolation bias)
SCHRAUD_SCALE = (128.0 / np.log(2.0)) * 0.125
SCHRAUD_BIAS = 16256.0 - 7.0

B, T, D, H = 2, 4096, 512, 8
HD = D // H  # 64
SW = 512  # q-slice width
NS = T // SW  # 8 q-slices
NKC = D // 128  # 4 contraction chunks for the projections
NTT = T // 128  # 32 t-tiles / key blocks
GK = 2  # key blocks grouped per exp() call (2 PSUM banks)
NEG = -1.0e9


def _split_waits(nc, max_waits=1):
    """The staged walrus rejects >1 semaphore wait per instruction; hoist
    extras onto same-engine NoOps inserted right before the instruction."""
    counter = 0
    for f in nc.m.functions:
        for blk in f.blocks:
            insts = blk.instructions
            out, changed = [], False
            for ins in insts:
                si = getattr(ins, "sync_info", None)
                waits = list(si.on_wait) if si is not None and si.on_wait else []
                if len(waits) > max_waits:
                    changed = True
                    for w in waits[:-max_waits]:
                        counter += 1
                        nop = mybir.InstNoOp(
                            name=f"I-wsplit-{counter}",
                            engine=ins.engine,
                            ins=[],
                            outs=[],
                        )
                        nop.sync_info = mybir.SyncInfo(on_wait=[w], on_update=[])
                        out.append(nop)
                    ins.sync_info = mybir.SyncInfo(
                        on_wait=waits[-max_waits:], on_update=list(si.on_update)
                    )
                out.append(ins)
            if changed:
                blk.instructions = out
    return counter


def build_nc():
    nc = bass.Bass("TRN2")

    xt = nc.dram_tensor("xt", [D, T], BF16, kind="ExternalInput")
    # w{q,k,v} host-packed [128, NKC*128]: chunk c of the [D,128] column
    # slice lives at cols [c*128,(c+1)*128)
    wq = nc.dram_tensor("wq", [128, D], BF16, kind="ExternalInput")
    wk = nc.dram_tensor("wk", [128, D], BF16, kind="ExternalInput")
    wv = nc.dram_tensor("wv", [128, D], BF16, kind="ExternalInput")
    wo = nc.dram_tensor("wo", [128, D], BF16, kind="ExternalInput")
    bq = nc.dram_tensor("bq", [128, 1], F32, kind="ExternalInput")
    bk = nc.dram_tensor("bk", [128, 1], F32, kind="ExternalInput")
    bv = nc.dram_tensor("bv", [1, 128], BF16, kind="ExternalInput")
    out = nc.dram_tensor("out", [T, D], F32, kind="ExternalOutput")

    ident_np = np.eye(128, dtype=np.float32)
    # maskneg[k, q'] = 0 where q' >= k else NEG  (S^T diagonal subtile mask)
    mask_np = np.where(
        np.arange(128)[None, :] >= np.arange(128)[:, None], 0.0, NEG
    ).astype(np.float32)
    ident_dram = nc.inline_tensor(ident_np.astype(_BF16), name="identc")
    mask_dram = nc.inline_tensor(mask_np.astype(_BF16), name="maskc")

    with TileContext(nc) as tc:
        with (
            tc.tile_pool(name="singles", bufs=1) as singles,
            tc.tile_pool(name="ps", bufs=3, space="PSUM") as ps,
            tc.tile_pool(name="zps", bufs=1, space="PSUM") as zps,
            tc.tile_pool(name="pt", bufs=6) as ptp,
            tc.tile_pool(name="sl", bufs=3) as slp,
            tc.tile_pool(name="outp", bufs=6) as outp,
            tc.tile_pool(name="drp", bufs=2, space="DRAM") as drp,
        ):
            # ---- static SBUF tiles ----
            xt_sb = [
                [
                    singles.tile(
                        [128, SW], BF16, tag=f"xt{c}_{s}", name=f"xt_sb{c}_{s}"
                    )
                    for s in range(NS)
                ]
                for c in range(NKC)
            ]
            wq_sb = singles.tile([128, D], BF16, tag="wq")
            wk_sb = singles.tile([128, D], BF16, tag="wk")
            wv_sb = singles.tile([128, D], BF16, tag="wv")
            wo_sb = singles.tile([128, D], BF16, tag="wo")
            bq_sb = singles.tile([128, 1], F32, tag="bq")
            bk_sb = singles.tile([128, 1], F32, tag="bk")
            bv_sb = singles.tile([128, 128], BF16, tag="bv")
            ident_sb = singles.tile([128, 128], BF16, tag="ident")
            mask_sb = singles.tile([128, 128], BF16, tag="mask")
            wob0_sb = singles.tile([HD, D], BF16, tag="wob0")

            # A single DMA runs on one DMA engine (~50GB/s): bandwidth needs
            # MANY in-flight transfers, and each queue issues one per
            # ~0.6us. Spread the 32 xt tiles + weights over three queues,
            # ordered by the descending schedule's consumption: slice 7
            # first, then 0, 1, 2, ...
            def xt_dma(eng, s):
                for c in range(NKC):
                    eng.dma_start(
                        out=xt_sb[c][s][:, :],
                        in_=xt[c * 128 : (c + 1) * 128, s * SW : (s + 1) * SW],
                    )

            xt_dma(nc.sync, NS - 1)
            nc.sync.dma_start(out=wq_sb[:, :], in_=wq[:, :])
            nc.sync.dma_start(out=wk_sb[:, :], in_=wk[:, :])
            for s in (0, 1):
                xt_dma(nc.sync, s)
            nc.scalar.dma_start(out=bq_sb[:, :], in_=bq[:, :])
            nc.scalar.dma_start(out=bk_sb[:, :], in_=bk[:, :])
            nc.scalar.dma_start(out=wv_sb[:, :], in_=wv[:, :])
            # bv replicated across partitions (step-0 partition AP, DRAM src)
            bvap = bv[:, :]
            nc.scalar.dma_start(
                out=bv_sb[:, :],
                in_=bass.AP(
                    tensor=bvap.tensor,
                    offset=bvap.offset,
                    ap=[[0, 128]] + list(bvap.ap[1:]),
                ),
            )
            xt_dma(nc.scalar, 2)
            for s in range(3, NS - 1):
                xt_dma(nc.gpsimd, s)
            nc.gpsimd.dma_start(out=ident_sb[:, :], in_=ident_dram[:, :])
            nc.gpsimd.dma_start(out=mask_sb[:, :], in_=mask_dram[:, :])
            nc.gpsimd.dma_start(out=wo_sb[:, :], in_=wo[:, :])
            # head-B rows of W_O re-homed at partitions 0..63 for the
            # last slice's per-head O-projection
            nc.gpsimd.dma_start(out=wob0_sb[:, :], in_=wo[HD:128, :])

            # preload the Exp activation table while DMAs land, so the
            # first real exp doesn't pay the 1.3us table load
            warm = singles.tile([1, 1], F32, tag="warm")
            nc.vector.memset(warm[:, :], 0.0)
            nc.scalar.activation(
                out=warm[:, :],
                in_=warm[:, :],
                func=mybir.ActivationFunctionType.Exp,
            )

            qt_sb = [
                singles.tile([128, SW], BF16, tag=f"qt{s}", name=f"qt_sb{s}")
                for s in range(NS)
            ]
            kt_sb = [
                singles.tile([128, SW], BF16, tag=f"kt{s}", name=f"kt_sb{s}")
                for s in range(NS)
            ]
            # V_aug per key block: [128(t), VA(64) | 1 | VB(64) | 1]
            v_sb = [
                singles.tile([128, 2 * HD + 2], BF16, tag=f"v{t}", name=f"v_sb{t}")
                for t in range(NTT)
            ]

            # ---- emission helpers ----
            def emit_qk(s):
                ps_q = ps.tile([128, SW], F32, tag="sg", name="ps_q")
                for c in range(NKC):
                    nc.tensor.matmul(
                        ps_q[:, :],
                        lhsT=wq_sb[:, c * 128 : (c + 1) * 128],
                        rhs=xt_sb[c][s][:, :],
                        start=(c == 0),
                        stop=(c == NKC - 1),
                        skip_group_check=True,
                    )
                nc.vector.tensor_scalar_add(qt_sb[s][:, :], ps_q[:, :], bq_sb[:, :])
                ps_k = ps.tile([128, SW], F32, tag="sg", name="ps_k")
                for c in range(NKC):
                    nc.tensor.matmul(
                        ps_k[:, :],
                        lhsT=wk_sb[:, c * 128 : (c + 1) * 128],
                        rhs=xt_sb[c][s][:, :],
                        start=(c == 0),
                        stop=(c == NKC - 1),
                        skip_group_check=True,
                    )
                nc.vector.tensor_scalar_add(kt_sb[s][:, :], ps_k[:, :], bk_sb[:, :])

            def emit_v(s):
                ps_v = ps.tile([128, 2 * SW], F32, tag="sg", name="ps_v")
                for t in range(4 * s, 4 * s + 4):
                    tloc = slice((t % 4) * 128, (t % 4 + 1) * 128)
                    vcol = slice((t % 4) * 128, (t % 4) * 128 + 128)
                    for c in range(NKC):
                        nc.tensor.matmul(
                            ps_v[:, vcol],
                            lhsT=xt_sb[c][s][:, tloc],
                            rhs=wv_sb[:, c * 128 : (c + 1) * 128],
                            start=(c == 0),
                            stop=(c == NKC - 1),
                            skip_group_check=True,
                        )
                for t in range(4 * s, 4 * s + 4):
                    c0 = (t % 4) * 128
                    # fused +b_V during evacuation; ones cols at 64 and 129
                    nc.vector.scalar_tensor_tensor(
                        v_sb[t][:, 0:HD],
                        ps_v[:, c0 : c0 + HD],
                        1.0,
                        bv_sb[:, 0:HD],
                        op0=mybir.AluOpType.mult,
                        op1=mybir.AluOpType.add,
                    )
                    nc.vector.scalar_tensor_tensor(
                        v_sb[t][:, HD + 1 : 2 * HD + 1],
                        ps_v[:, c0 + HD : c0 + 128],
                        1.0,
                        bv_sb[:, HD:128],
                        op0=mybir.AluOpType.mult,
                        op1=mybir.AluOpType.add,
                    )
                    nc.vector.memset(v_sb[t][:, HD : HD + 1], 1.0)
                    nc.vector.memset(v_sb[t][:, 2 * HD + 1 : 2 * HD + 2], 1.0)

            vcols = (slice(0, HD + 1), slice(HD + 1, 2 * HD + 2))
            hrows = (slice(0, HD), slice(HD, 128))

            def emit_oproj(znpair_t, qs_t):
                for j in range(4):
                    ps_o = ps.tile([128, 2 * SW], F32, tag="sg", name="ps_o")
                    nc.tensor.matmul(
                        ps_o[:, 0:D],
                        lhsT=znpair_t[:, j * 128 : (j + 1) * 128],
                        rhs=wo_sb[:, :],
                        start=True,
                        stop=True,
                        skip_group_check=True,
                    )
                    o_sb = outp.tile([128, D], F32, tag="ot", name="o_sb")
                    nc.vector.tensor_copy(o_sb[:, :], ps_o[:, 0:D])
                    r0 = qs_t + j * 128
                    nc.sync.dma_start(out=out[r0 : r0 + 128, :], in_=o_sb[:, :])

            # ---- main loop. Slices are processed in DESCENDING order
            # (7,6,...,0): slice 7 touches every key block, so all QKV
            # emissions interleave between its 16 score groups, exp work is
            # front-loaded for ScalarE, and the kernel ends on the smallest
            # slice where the output tail is cheapest. The scores->exp->PV
            # chain is pipelined ACROSS slices: the previous slice's last PV
            # + normalisation + O-projections are emitted under the next
            # slice's first score groups. ----
            emit_qk(NS - 1)
            emit_qk(0)
            pending = []
            av_queue = []  # (pt, grp, zaug, nkb, qs)

            def emit_av(av):
                pt_t, h1off, kb, n, qlo, zaug_t, nkb_t, qs_t = av
                for h in range(2):
                    rhs_ap = pt_t[:, h * h1off : h * h1off + n]
                    nc.tensor.matmul(
                        zaug_t[h][0 : HD + 1, qlo - qs_t : SW],
                        lhsT=v_sb[kb][:, vcols[h]],
                        rhs=rhs_ap,
                        start=(kb == 0),
                        stop=(kb == nkb_t - 1),
                        skip_group_check=True,
                    )

            def emit_norm(zaug, qs):
                # normalisation: evacuate Z_aug (frees the PSUM bank),
                # 1/L on a [128,8] partition-spread via one DRAM round-trip
                # shared by both heads, broadcast back with a step-0
                # partition DMA read (legal from DRAM), one multiply
                zsb = [None, None]
                for h in range(2):
                    zsb[h] = slp.tile([HD + 1, SW], F32, tag=f"zsb{h}", name="zsb")
                    nc.vector.tensor_copy(zsb[h][:, :], zaug[h][:, :])
                rd = drp.tile([1, 2 * SW], F32, tag="rd", name="rd")
                for h in range(2):
                    nc.sync.dma_start(
                        out=rd[:, h * SW : (h + 1) * SW], in_=zsb[h][HD : HD + 1, :]
                    )
                lsp = slp.tile([128, 2 * SW // 128], F32, tag="lsp", name="lsp")
                nc.sync.dma_start(
                    out=lsp[:, :], in_=rd[0, :].rearrange("(p f) -> p f", p=128)
                )
                rsp = slp.tile([128, 2 * SW // 128], F32, tag="rsp", name="rsp")
                nc.vector.reciprocal(rsp[:, :], lsp[:, :])
                rd2 = drp.tile([1, 2 * SW], F32, tag="rd2", name="rd2")
                nc.sync.dma_start(
                    out=rd2[0, :].rearrange("(p f) -> p f", p=128), in_=rsp[:, :]
                )
                bc_sb = slp.tile([HD, 2 * SW], F32, tag="bcs", name="bc_sb")
                rap = rd2[:, :]
                nc.sync.dma_start(
                    out=bc_sb[:, :],
                    in_=bass.AP(
                        tensor=rap.tensor,
                        offset=rap.offset,
                        ap=[[0, HD]] + list(rap.ap[1:]),
                    ),
                )
                znpair = slp.tile([128, SW], BF16, tag="zn")
                znb = slp.tile([HD, SW], BF16, tag="znb")
                for h in range(2):
                    dst = znpair[0:HD, :] if h == 0 else znb[:, :]
                    nc.vector.tensor_mul(
                        dst, zsb[h][0:HD, :], bc_sb[:, h * SW : (h + 1) * SW]
                    )
                # move head B rows into partitions 64..127
                nc.gpsimd.dma_start(out=znpair[HD:128, :], in_=znb[:, :])
                pending.append((znpair, qs))

            prev = None  # previous slice's (zaug, qs) awaiting normalisation
            for si, s in enumerate([7, 6, 5, 4, 3, 2, 1, 0]):
                qs = s * SW
                nkb = 4 * (s + 1)
                zaug = [
                    zps.tile([HD + 1, SW], F32, tag="za", name="zauga"),
                    zps.tile([HD + 1, SW], F32, tag="zb", name="zaugb"),
                ]
                # one PSUM tile per key block holds BOTH heads (h0 at cols
                # 0:SW, h1 at SW:2SW): the head-pair's score matmuls are
                # same-engine writes to disjoint regions of one tile — no
                # semaphore between them, so the PE's 64-deep window can
                # run the two row-disjoint (tile_position) matmuls
                # concurrently; one exp covers both heads
                for kb in range(nkb):
                    qlo = max(qs, kb * 128)
                    n = qs + SW - qlo
                    diag = kb * 128 >= qs
                    # head-1 packs right after head-0 when the block fits in
                    # one PSUM bank (no matmul output may straddle a bank):
                    # saves exp-ing the stale [n, SW) sliver of narrow
                    # diagonal blocks
                    h1off = SW
                    sg = ps.tile([128, 2 * SW], F32, tag="sg", name="sg")
                    for h in range(2):
                        nc.tensor.matmul(
                            sg[:, h * h1off : h * h1off + n],
                            lhsT=kt_sb[kb // 4][
                                hrows[h], (kb % 4) * 128 : (kb % 4 + 1) * 128
                            ],
                            rhs=qt_sb[s][hrows[h], qlo - qs : qlo - qs + n],
                            start=True,
                            stop=not diag,
                            skip_group_check=True,
                            tile_position=(h * HD, 0),
                        )
                    if diag:
                        for h in range(2):
                            nc.tensor.matmul(
                                sg[:, h * h1off : h * h1off + 128],
                                lhsT=ident_sb[:, :],
                                rhs=mask_sb[:, :],
                                start=False,
                                stop=True,
                                skip_group_check=True,
                            )
                    pt = ptp.tile([128, 2 * SW], BF16, tag="pt", name="pt")
                    # one exp over [0, SW+n): for diag blocks the stale
                    # [n, SW) sliver is exp'd too (never read; harmless)
                    nc.scalar.activation(
                        out=pt[:, 0 : h1off + n],
                        in_=sg[:, 0 : h1off + n],
                        func=mybir.ActivationFunctionType.Exp,
                        scale=0.125,
                    )
                    av_queue.append((pt, h1off, kb, n, qlo, zaug, nkb, qs))
                    while len(av_queue) > 1:
                        emit_av(av_queue.pop(0))
                    if si == 0:
                        # slice 7 (processed first) consumes kt/V of slice
                        # kb//4 at block kb: spread the whole model's QKV
                        # emissions a couple of blocks ahead of their first
                        # use, so the PE stream is useful from the first
                        # instruction and exp starts at t~5us
                        if kb % 4 == 0:
                            emit_v(kb // 4)
                        if kb % 4 == 2 and kb // 4 + 1 < NS - 1:
                            emit_qk(kb // 4 + 1)
                    else:
                        # spread the non-score work across the slice's first
                        # blocks (keeps ScalarE's exp stream from starving
                        # at slice boundaries)
                        if kb == 0 and prev is not None:
                            pz, pq = prev
                            prev = None
                            emit_norm(pz, pq)
                        if kb == 1:
                            while len(pending) > 1:
                                emit_oproj(*pending.pop(0))
                prev = (zaug, qs)

            # ---- tail: last slice's PV + its L-bounce start immediately;
            # the remaining O-projection overlaps the bounce latency; then
            # the per-head O-projection scaled at PSUM eviction by the
            # partition-spread reciprocal (no broadcast, no head-B shift) ----
            while av_queue:
                emit_av(av_queue.pop(0))
            zaug, qs = prev
            znu = [None, None]
            lr2 = slp.tile([1, 2 * SW], F32, tag="lr2", name="lr2")
            for h in range(2):
                nc.vector.tensor_copy(
                    lr2[:, h * SW : (h + 1) * SW], zaug[h][HD : HD + 1, :]
                )
            rdl = drp.tile([1, 2 * SW], F32, tag="rdl", name="rdl")
            nc.gpsimd.dma_start(out=rdl[:, :], in_=lr2[:, :])
            lspl = slp.tile([128, 2 * SW // 128], F32, tag="lsp", name="lspl")
            # column-major spread: lspl[p, f] = L[f*128 + p], so
            # rspl[:, j] is exactly 1/L for q-subtile j, per-partition
            nc.gpsimd.dma_start(
                out=lspl[:, :], in_=rdl[0, :].rearrange("(f p) -> p f", p=128)
            )
            for h in range(2):
                znu[h] = slp.tile([HD, SW], BF16, tag=f"znu{h}", name="znu")
                nc.vector.tensor_copy(znu[h][:, :], zaug[h][0:HD, :])
            while pending:
                emit_oproj(*pending.pop(0))
            rspl = slp.tile([128, 2 * SW // 128], F32, tag="rsp", name="rspl")
            nc.vector.reciprocal(rspl[:, :], lspl[:, :])
            for j in range(4):
                ps_a = ps.tile([128, 2 * SW], F32, tag="sg", name="ps_oa")
                nc.tensor.matmul(
                    ps_a[:, 0:D],
                    lhsT=znu[0][:, j * 128 : (j + 1) * 128],
                    rhs=wo_sb[0:HD, :],
                    start=True,
                    stop=True,
                    skip_group_check=True,
                )
                ps_b = ps.tile([128, 2 * SW], F32, tag="sg", name="ps_ob")
                nc.tensor.matmul(
                    ps_b[:, 0:D],
                    lhsT=znu[1][:, j * 128 : (j + 1) * 128],
                    rhs=wob0_sb[:, :],
                    start=True,
                    stop=True,
                    skip_group_check=True,
                )
                o_sb = outp.tile([128, D], F32, tag="ot", name="o_sb")
                nc.scalar.activation(
                    out=o_sb[:, :],
                    in_=ps_a[:, 0:D],
                    func=mybir.ActivationFunctionType.Copy,
                    scale=rspl[:, j : j + 1],
                )
                nc.vector.scalar_tensor_tensor(
                    o_sb[:, :],
                    ps_b[:, 0:D],
                    rspl[:, 4 + j : 5 + j],
                    o_sb[:, :],
                    op0=mybir.AluOpType.mult,
                    op1=mybir.AluOpType.add,
                )
                r0 = qs + j * 128
                nc.sync.dma_start(out=out[r0 : r0 + 128, :], in_=o_sb[:, :])

    _split_waits(nc)
    return nc


_NC_CACHE = {}


def _get_nc():
    if "nc" not in _NC_CACHE:
        _NC_CACHE["nc"] = build_nc()
    return _NC_CACHE["nc"]


def make_in_maps(combined_embed, W_K, b_K, W_Q, b_Q, W_V, b_V, W_O, b_O):
    f32 = np.float32

    def packw(W, sl):
        # [D,128] column slice -> [128, NKC*128] with chunk c at cols c*128+
        Wc = np.asarray(W, f32)[:, sl]
        return np.ascontiguousarray(
            np.concatenate([Wc[c * 128 : (c + 1) * 128, :] for c in range(NKC)], 1)
        ).astype(_BF16)

    in_maps = []
    for c in range(8):
        b = c // 4
        g = c % 4
        sl = slice(g * 128, (g + 1) * 128)
        xt = np.ascontiguousarray(np.asarray(combined_embed[b], f32).T)
        in_maps.append(
            {
                "xt": xt.astype(_BF16),
                "wq": packw(W_Q, sl),
                "wk": packw(W_K, sl),
                "wv": packw(W_V, sl),
                "wo": np.ascontiguousarray(np.asarray(W_O, f32)[sl, :]).astype(_BF16),
                "bq": np.asarray(b_Q, f32)[sl].reshape(128, 1).copy(),
                "bk": np.asarray(b_K, f32)[sl].reshape(128, 1).copy(),
                "bv": np.asarray(b_V, f32)[sl].reshape(1, 128).astype(_BF16),
            }
        )
    return in_maps


def run_cores(in_maps, **kwargs):
    nc = _get_nc()
    return run_bass_kernel_spmd(nc, in_maps, core_ids=list(range(8)), **kwargs)


def kernel(
    combined_embed, W_K, b_K, W_Q, b_Q, W_V, b_V, W_O, b_O
):  # full inputs -> full output
    in_maps = make_in_maps(
        combined_embed, W_K, b_K, W_Q, b_Q, W_V, b_V, W_O, b_O
    )
    res = run_cores(in_maps)
    out = np.zeros((B, T, D), np.float32)
    for c in range(8):
        out[c // 4] += res.results[c]["out"]
    out += np.asarray(b_O, np.float32)[None, None, :]
    return out
